# revision 1
# baseline (speedup 1.0000x reference)
"""BiMPNN layer on 8 Trainium2 NeuronCores (Bass/Tile).

Math (reassociated from the reference):
    out = gelu( (A h) @ W^T + (A^T h) @ Wt^T + h @ Ws^T
                + deg_out x W_b + deg_in x Wt_b + Ws_b )
where A is the COO adjacency.  Aggregating raw node features first means
the gather/segment-sum operates on h directly, and each dense projection
runs on already-aggregated per-node data.

Distribution: nodes are bin-packed into 784 destination tiles of 128
slots (balancing per-tile edge counts in both directions); core c owns
98 tiles.  h and the small weights are replicated per core; each core
receives only its own edge metadata.  No collectives.

Device pipeline per destination tile (128 dest slots):
  - dma_gather (Q7 ucode, int16 idx) pulls all inbound-edge source rows,
    one 512B row per edge, bucketed into <=32768-row sub-tables of h so
    indices fit int16; per (supertile, bucket) one big call.
  - per-tile self rows come via a 128-row indirect DMA (int32 offsets).
  - per 128-edge chunk: sel = is_equal(dest_slot, iota_row) on DVE, then
    PE matmul psum[feat,dest] += chunk.T @ sel (PSUM f32 accumulation)
    -> builds G1^T, G2^T and h_own^T with no explicit transposes.
  - dense: psum_out[dout,dest] = W^T.T@G1T + Wt^T.T@G2T + Ws^T.T@hT
           + bias3.T @ [deg1;deg2;ones]   (K=3 rank-3 bias matmul)
  - exact-erf GELU on ACT -> output staged [dout, dest];
    host transposes and un-permutes.
"""

import hashlib
import json

import numpy as np

import concourse.bass as bass
import concourse.mybir as mybir
import concourse.tile as tile
import concourse.bass_utils as bass_utils
import concourse.bass2jax as bass2jax
from concourse import library_config
from concourse.tile_rust import add_dep_helper
from concourse.bass_utils import run_bass_kernel_spmd

# ---------------------------------------------------------------------------
# BIR fixup: this walrus build lowers at most ONE sync wait per instruction
# ("Too many sync wait commands").  Hoist excess waits onto same-engine NoOps
# inserted immediately before the offending instruction (per-engine program
# order is preserved, so the waits still complete before the instruction).
_MAX_WAITS = 1


def _split_excess_waits(bir_json: bytes) -> bytes:
    m = json.loads(bir_json)
    ctr = 0
    changed = False
    for fn in m["functions"]:
        for blk in fn["blocks"]:
            new_insts = []
            for inst in blk["instructions"]:
                body = inst
                if len(inst) == 1 and isinstance(next(iter(inst.values())), dict):
                    body = inst[next(iter(inst))]
                si = body.get("sync_info") if isinstance(body, dict) else None
                waits = si.get("on_wait") if si else None
                if waits and len(waits) > _MAX_WAITS:
                    changed = True
                    excess, keep = waits[:-_MAX_WAITS], waits[-_MAX_WAITS:]
                    while excess:
                        part, excess = excess[:_MAX_WAITS], excess[_MAX_WAITS:]
                        ctr += 1
                        new_insts.append({
                            "debug": body.get("debug", 0),
                            "engine": body.get("engine"),
                            "ins": [], "outs": [],
                            "name": f"I-waitsplit-{ctr}",
                            "opcode": "NoOp",
                            "sync_info": {"on_update": [], "on_wait": part},
                        })
                    si["on_wait"] = keep
                new_insts.append(inst)
            blk["instructions"] = new_insts
    if changed:
        return json.dumps(m).encode()
    return bir_json


if not getattr(bass_utils, "_waitsplit_patched", False):
    _orig_compile_bir_kernel = bass_utils.compile_bir_kernel

    def _patched_compile_bir_kernel(bir_json, tmpdir, neff_name="file.neff"):
        return _orig_compile_bir_kernel(
            _split_excess_waits(bir_json), tmpdir, neff_name)

    bass_utils.compile_bir_kernel = _patched_compile_bir_kernel
    bass2jax.compile_bir_kernel = _patched_compile_bir_kernel
    bass_utils._waitsplit_patched = True

# ---------------------------------------------------------------------------

F32 = mybir.dt.float32
I32 = mybir.dt.int32
I16 = mybir.dt.int16

SUBTAB = 32768   # dma_gather idx is int16: sub-tables of <= 32768 rows

FULL_CFG = dict(N=100000, D=128, NCORES=8, TPC=98, SUP=2)


def _bounds(N, subtab=SUBTAB):
    b = list(range(0, N, subtab)) + [N]
    return b


# ---------------------------------------------------------------------------
# Host-side preprocessing
def pack_graph(rows, cols, cfg):
    N, NCORES, TPC, SUP = cfg["N"], cfg["NCORES"], cfg["TPC"], cfg["SUP"]
    NT = NCORES * TPC
    NSUP = TPC // SUP
    SLOTS = NT * 128
    assert SLOTS >= N and TPC % SUP == 0
    bounds = _bounds(N, cfg.get("subtab", SUBTAB))
    NB = len(bounds) - 1

    rows = np.asarray(rows).astype(np.int64)
    cols = np.asarray(cols).astype(np.int64)
    E = rows.shape[0]
    deg1 = np.bincount(rows, minlength=N)
    deg2 = np.bincount(cols, minlength=N)

    # --- balanced bin packing (LPT-style snake over combined degree) ----
    w = deg1 + deg2
    order = np.argsort(-w, kind="stable")
    bin_of_node = np.full(N, -1, np.int32)
    loads = np.zeros(NT, np.int64)
    counts = np.zeros(NT, np.int64)
    pos = 0
    while pos < N:
        free = np.where(counts < 128)[0]
        take = min(free.shape[0], N - pos)
        tgt = free[np.argsort(loads[free], kind="stable")][:take]
        nodes = order[pos:pos + take]
        bin_of_node[nodes] = tgt
        loads[tgt] += w[nodes]
        counts[tgt] += 1
        pos += take

    # slots within each tile
    node_of_slot = np.full(SLOTS, -1, np.int64)
    o2 = np.argsort(bin_of_node, kind="stable")
    srt_bins = bin_of_node[o2]
    starts = np.searchsorted(srt_bins, np.arange(NT))
    rank_in_tile = np.zeros(N, np.int64)
    rank_in_tile[o2] = np.arange(N) - starts[srt_bins]
    slot_of_node = bin_of_node.astype(np.int64) * 128 + rank_in_tile
    node_of_slot[slot_of_node] = np.arange(N)

    buck_of = np.searchsorted(bounds, np.arange(N), side="right") - 1

    def lay(dest, src):
        """Per-direction layout: idx arrays (int16, wrapped+replicated)
        and dest-slot arrays, per core."""
        t_e = bin_of_node[dest].astype(np.int64)
        b_e = buck_of[src]
        key = t_e * NB + b_e
        o = np.argsort(key, kind="stable")
        ks = key[o]
        cnt = np.bincount(ks, minlength=NT * NB)
        start = np.concatenate([[0], np.cumsum(cnt)[:-1]])
        r = np.arange(E) - start[ks]
        kb = np.ceil(cnt.reshape(NT, NB).max(axis=0) / 128).astype(int)
        off = np.concatenate([[0], np.cumsum(kb)[:-1]])
        Ktot = int(kb.sum())

        gx = np.zeros((NCORES, 128, NSUP * Ktot * SUP * 8), np.int16)
        dl = np.full((NCORES, 128, TPC * Ktot), -1.0, np.float32)

        t = ks // NB
        b = ks % NB
        core = t // TPC
        tl = t % TPC
        sup = tl // SUP
        tin = tl % SUP
        chunk = r // 128
        p = r % 128
        assert (chunk < kb[b]).all()
        # dest-slot value array (matches ged column layout per sup block)
        colblock = off[b] * SUP + tin * kb[b] + chunk
        dl[core, p, sup * (SUP * Ktot) + colblock] = \
            (slot_of_node[dest[o]] % 128).astype(np.float32)
        # idx value (sub-table local), wrapped [16 x cols] + replicated x8
        i_call = (tin * kb[b] + chunk) * 128 + p
        colbase = sup * (Ktot * SUP * 8) + off[b] * (SUP * 8)
        val = (src[o] - np.asarray(bounds)[b]).astype(np.int16)
        gx4 = gx.reshape(NCORES, 8, 16, NSUP * Ktot * SUP * 8)
        gx4[core, :, i_call % 16, colbase + i_call // 16] = val[:, None]
        return kb, off, Ktot, gx, dl

    kb1, off1, K1, gx1, dl1 = lay(rows, cols)
    kb2, off2, K2, gx2, dl2 = lay(cols, rows)

    # self rows: per-slot node id (int32) + dest-slot value
    ns = node_of_slot.reshape(NCORES, TPC, 128)
    gsl = np.where(ns >= 0, ns, 0).astype(np.int32).transpose(0, 2, 1).copy()
    dsl = np.where(ns >= 0, np.arange(128)[None, None, :], -1) \
        .astype(np.float32).transpose(0, 2, 1).copy()

    degs = np.zeros((NCORES, 3, TPC * 128), np.float32)
    for c in range(NCORES):
        sl = node_of_slot[c * TPC * 128:(c + 1) * TPC * 128]
        valid = sl >= 0
        svl = np.where(valid, sl, 0)
        degs[c, 0] = np.where(valid, deg1[svl], 0).astype(np.float32)
        degs[c, 1] = np.where(valid, deg2[svl], 0).astype(np.float32)
        degs[c, 2] = valid.astype(np.float32)

    return dict(kb1=tuple(int(x) for x in kb1), kb2=tuple(int(x) for x in kb2),
                K1=K1, K2=K2, node_of_slot=node_of_slot,
                gx1=gx1, dl1=dl1, gx2=gx2, dl2=dl2,
                gsl=gsl, dsl=dsl, degs=degs)


# ---------------------------------------------------------------------------
# Device program
def build_nc(cfg, kb1, kb2):
    N, D, TPC, SUP = cfg["N"], cfg["D"], cfg["TPC"], cfg["SUP"]
    NSUP = TPC // SUP
    bounds = _bounds(N, cfg.get("subtab", SUBTAB))
    NB = len(bounds) - 1
    assert len(kb1) == len(kb2) == NB
    K1, K2 = sum(kb1), sum(kb2)
    off1 = np.concatenate([[0], np.cumsum(kb1)[:-1]]).astype(int)
    off2 = np.concatenate([[0], np.cumsum(kb2)[:-1]]).astype(int)
    SLOTS = TPC * 128

    nc = bass.Bass()
    h = nc.declare_dram_parameter("h", [N, D], F32, isOutput=False)
    gx1 = nc.declare_dram_parameter("gx1", [128, NSUP * K1 * SUP * 8], I16,
                                    isOutput=False)
    dl1 = nc.declare_dram_parameter("dl1", [128, TPC * K1], F32, isOutput=False)
    gx2 = nc.declare_dram_parameter("gx2", [128, NSUP * K2 * SUP * 8], I16,
                                    isOutput=False)
    dl2 = nc.declare_dram_parameter("dl2", [128, TPC * K2], F32, isOutput=False)
    gsl = nc.declare_dram_parameter("gsl", [128, TPC], I32, isOutput=False)
    dsl = nc.declare_dram_parameter("dsl", [128, TPC], F32, isOutput=False)
    wT = nc.declare_dram_parameter("wT", [D, 3 * D], F32, isOutput=False)
    b3 = nc.declare_dram_parameter("b3", [3, D], F32, isOutput=False)
    iotam = nc.declare_dram_parameter("iotam", [128, 128], F32, isOutput=False)
    degs = nc.declare_dram_parameter("degs", [3, SLOTS], F32, isOutput=False)
    out = nc.declare_dram_parameter("out", [D, SLOTS], F32, isOutput=True)

    with tile.TileContext(nc) as tc:
        with (
            tc.tile_pool(name="const", bufs=1) as cpool,
            tc.tile_pool(name="aux", bufs=2) as apool,
            tc.tile_pool(name="ged", bufs=2) as gpool,
            tc.tile_pool(name="work", bufs=4) as wpool,
            tc.tile_pool(name="stage", bufs=2) as spool,
            tc.tile_pool(name="psum", bufs=2, space="PSUM") as ppool,
        ):
            lib = nc.gpsimd.load_library(library_config.mlp)

            # one register per distinct num_idxs constant (to_reg inside
            # dma_gather would otherwise exhaust Pool registers)
            _regs = {}

            def nidx_reg(v):
                if v not in _regs:
                    _regs[v] = nc.gpsimd.to_reg(v)
                return _regs[v]

            iota_f = cpool.tile([128, 128], F32)
            nc.sync.dma_start(out=iota_f[:], in_=iotam[:])
            wT_sb = cpool.tile([D, 3 * D], F32)
            nc.sync.dma_start(out=wT_sb[:], in_=wT[:])
            b3_sb = cpool.tile([3, D], F32)
            nc.sync.dma_start(out=b3_sb[:], in_=b3[:])

            import contextlib
            rep_ctx = (tc.For_i(0, cfg["repeat"], 1)
                       if cfg.get("repeat", 1) > 1 else
                       contextlib.nullcontext())
            with rep_ctx:
              for sup in range(NSUP):
                  t0 = sup * SUP
                  gx1_sb = apool.tile([128, K1 * SUP * 8], I16)
                  dl1_sb = apool.tile([128, SUP * K1], F32)
                  gx2_sb = apool.tile([128, K2 * SUP * 8], I16)
                  dl2_sb = apool.tile([128, SUP * K2], F32)
                  gsl_sb = apool.tile([128, SUP], I32)
                  dsl_sb = apool.tile([128, SUP], F32)
                  degs_sb = apool.tile([3, SUP * 128], F32)
                  nc.sync.dma_start(
                      out=gx1_sb[:],
                      in_=gx1[:, sup * K1 * SUP * 8:(sup + 1) * K1 * SUP * 8])
                  nc.sync.dma_start(
                      out=dl1_sb[:], in_=dl1[:, t0 * K1:(t0 + SUP) * K1])
                  nc.sync.dma_start(
                      out=gx2_sb[:],
                      in_=gx2[:, sup * K2 * SUP * 8:(sup + 1) * K2 * SUP * 8])
                  nc.sync.dma_start(
                      out=dl2_sb[:], in_=dl2[:, t0 * K2:(t0 + SUP) * K2])
                  nc.sync.dma_start(out=gsl_sb[:], in_=gsl[:, t0:t0 + SUP])
                  nc.sync.dma_start(out=dsl_sb[:], in_=dsl[:, t0:t0 + SUP])
                  nc.sync.dma_start(
                      out=degs_sb[:], in_=degs[:, t0 * 128:(t0 + SUP) * 128])
                  out_st = spool.tile([D, SUP * 128], F32)

                  ged1 = gpool.tile([128, SUP * K1, 128], F32)
                  ged2 = gpool.tile([128, SUP * K2, 128], F32)
                  for b in range(NB):
                      if kb1[b]:
                          g = nc.gpsimd.dma_gather(
                              out_ap=ged1[:, off1[b] * SUP:
                                          (off1[b] + kb1[b]) * SUP, :],
                              in_ap=h[bounds[b]:bounds[b + 1], :],
                              idxs_ap=gx1_sb[:, off1[b] * SUP * 8:
                                             (off1[b] + kb1[b]) * SUP * 8],
                              num_idxs=SUP * kb1[b] * 128,
                              num_idxs_reg=nidx_reg(SUP * kb1[b] * 128),
                              elem_size=D, single_packet=False)
                          add_dep_helper(g.ins, lib.ins, False, "lib first")
                      if kb2[b]:
                          g = nc.gpsimd.dma_gather(
                              out_ap=ged2[:, off2[b] * SUP:
                                          (off2[b] + kb2[b]) * SUP, :],
                              in_ap=h[bounds[b]:bounds[b + 1], :],
                              idxs_ap=gx2_sb[:, off2[b] * SUP * 8:
                                             (off2[b] + kb2[b]) * SUP * 8],
                              num_idxs=SUP * kb2[b] * 128,
                              num_idxs_reg=nidx_reg(SUP * kb2[b] * 128),
                              elem_size=D, single_packet=False)
                          add_dep_helper(g.ins, lib.ins, False, "lib first")

                  for ti in range(SUP):
                      ged_self = wpool.tile([128, 128], F32, tag="gself")
                      nc.gpsimd.indirect_dma_start(
                          out=ged_self[:], out_offset=None, in_=h[:, :],
                          in_offset=bass.IndirectOffsetOnAxis(
                              ap=gsl_sb[:, ti:ti + 1], axis=0))

                      # chunk columns of this tile in ged1/ged2
                      cols1 = [off1[b] * SUP + ti * kb1[b] + k
                               for b in range(NB) for k in range(kb1[b])]
                      cols2 = [off2[b] * SUP + ti * kb2[b] + k
                               for b in range(NB) for k in range(kb2[b])]

                      ps_g1 = ppool.tile([D, 128], F32, tag="ps_g1")
                      for j, col in enumerate(cols1):
                          sel = wpool.tile([128, 128], F32, tag="sel")
                          nc.vector.tensor_tensor(
                              out=sel[:],
                              in0=dl1_sb[:, col:col + 1]
                                  .to_broadcast([128, 128]),
                              in1=iota_f[:],
                              op=mybir.AluOpType.is_equal)
                          nc.tensor.matmul(
                              ps_g1[:], lhsT=ged1[:, col, :], rhs=sel[:],
                              start=(j == 0), stop=(j == len(cols1) - 1))
                      ps_ht = ppool.tile([D, 128], F32, tag="ps_ht")
                      sel = wpool.tile([128, 128], F32, tag="sel")
                      nc.vector.tensor_tensor(
                          out=sel[:],
                          in0=dsl_sb[:, ti:ti + 1].to_broadcast([128, 128]),
                          in1=iota_f[:], op=mybir.AluOpType.is_equal)
                      nc.tensor.matmul(ps_ht[:], lhsT=ged_self[:], rhs=sel[:],
                                       start=True, stop=True)
                      ps_g2 = ppool.tile([D, 128], F32, tag="ps_g2")
                      for j, col in enumerate(cols2):
                          sel = wpool.tile([128, 128], F32, tag="sel")
                          nc.vector.tensor_tensor(
                              out=sel[:],
                              in0=dl2_sb[:, col:col + 1]
                                  .to_broadcast([128, 128]),
                              in1=iota_f[:],
                              op=mybir.AluOpType.is_equal)
                          nc.tensor.matmul(
                              ps_g2[:], lhsT=ged2[:, col, :], rhs=sel[:],
                              start=(j == 0), stop=(j == len(cols2) - 1))

                      # move aggregates to SBUF for the dense projections
                      g1T = wpool.tile([D, 128], F32, tag="g1T")
                      g2T = wpool.tile([D, 128], F32, tag="g2T")
                      hT = wpool.tile([D, 128], F32, tag="hT")
                      nc.vector.tensor_copy(g1T[:], ps_g1[:])
                      nc.vector.tensor_copy(g2T[:], ps_g2[:])
                      nc.vector.tensor_copy(hT[:], ps_ht[:])

                      ps_out = ppool.tile([D, 128], F32, tag="ps_out")
                      nc.tensor.matmul(ps_out[:], lhsT=wT_sb[:, 0:D],
                                       rhs=g1T[:], start=True, stop=False)
                      nc.tensor.matmul(ps_out[:], lhsT=wT_sb[:, D:2 * D],
                                       rhs=g2T[:], start=False, stop=False)
                      nc.tensor.matmul(ps_out[:], lhsT=wT_sb[:, 2 * D:3 * D],
                                       rhs=hT[:], start=False, stop=False)
                      nc.tensor.matmul(ps_out[:], lhsT=b3_sb[:],
                                       rhs=degs_sb[:, ti * 128:(ti + 1) * 128],
                                       start=False, stop=True)

                      act = (mybir.ActivationFunctionType.Identity
                             if cfg.get("act") == "none"
                             else mybir.ActivationFunctionType.Gelu)
                      nc.scalar.activation(
                          out=out_st[:, ti * 128:(ti + 1) * 128], in_=ps_out[:],
                          func=act)

                  nc.sync.dma_start(
                      out=out[:, t0 * 128:(t0 + SUP) * 128], in_=out_st[:])

    mybir.codegen_inst_isa_subclasses(nc)
    return nc


# ---------------------------------------------------------------------------
_NC_CACHE = {}
_PREP_CACHE = {}


def _get_nc(cfg, kb1, kb2):
    key = (tuple(sorted(cfg.items())), kb1, kb2)
    if key not in _NC_CACHE:
        _NC_CACHE[key] = build_nc(cfg, kb1, kb2)
    return _NC_CACHE[key]


def make_in_maps(h_n, W_w, W_b, Wt_w, Wt_b, Ws_w, Ws_b, pk, cfg):
    h_np = np.ascontiguousarray(np.asarray(h_n), np.float32)
    wT = np.ascontiguousarray(
        np.concatenate([np.asarray(W_w).T, np.asarray(Wt_w).T,
                        np.asarray(Ws_w).T], axis=1), np.float32)
    b3 = np.ascontiguousarray(
        np.stack([np.asarray(W_b), np.asarray(Wt_b), np.asarray(Ws_b)]),
        np.float32)
    iota = np.tile(np.arange(128, dtype=np.float32), (128, 1))
    in_maps = []
    for c in range(cfg["NCORES"]):
        in_maps.append({
            "h": h_np,
            "gx1": pk["gx1"][c], "dl1": pk["dl1"][c],
            "gx2": pk["gx2"][c], "dl2": pk["dl2"][c],
            "gsl": pk["gsl"][c], "dsl": pk["dsl"][c],
            "wT": wT, "b3": b3, "iotam": iota, "degs": pk["degs"][c],
        })
    return in_maps


def run(h_n, W_w, W_b, Wt_w, Wt_b, Ws_w, Ws_b, rows, cols, cfg):
    N, D, NCORES, TPC = cfg["N"], cfg["D"], cfg["NCORES"], cfg["TPC"]
    key = hashlib.md5(
        np.ascontiguousarray(rows).tobytes()
        + np.ascontiguousarray(cols).tobytes()).hexdigest()
    if key not in _PREP_CACHE:
        _PREP_CACHE[key] = pack_graph(rows, cols, cfg)
    pk = _PREP_CACHE[key]
    nc = _get_nc(cfg, pk["kb1"], pk["kb2"])
    in_maps = make_in_maps(h_n, W_w, W_b, Wt_w, Wt_b, Ws_w, Ws_b, pk, cfg)
    res = run_bass_kernel_spmd(nc, in_maps, list(range(NCORES)))

    out_full = np.empty((N, D), np.float32)
    nos = pk["node_of_slot"]
    for c in range(NCORES):
        o = res.results[c]["out"]            # [D, TPC*128]
        sl = nos[c * TPC * 128:(c + 1) * TPC * 128]
        valid = sl >= 0
        out_full[sl[valid]] = o.T[valid]
    return out_full


def kernel(**inputs):
    return run(cfg=FULL_CFG, **inputs)

